# revision 26
# baseline (speedup 1.0000x reference)
"""ABMIL gated-attention MIL pooling on 8 TRN2 NeuronCores.

Work-item data parallelism: every 512-token group of every bag is an
independent work item; the ceil(G_tot/8) items per core are balanced
across cores.  Per item (512 tokens, D=1024, H=256):

    A   = tanh(x Vw + Vb) * sigmoid(x Uw + Ub)        [512, H]
    s   = A Ww                                        [512]
    e   = exp(s) * mask                               [512]   (no max-sub:
          |s| <= sum|0.5 W| ~ 13, exp fits f32/bf16 easily)
    zk  = e @ x_group,  dk = sum(e)                   [D], [1]

Host combines: Z_b = (sum_k zk) / (sum_k dk) over the bag's items.
Wb shifts every score equally -> cancels -> dropped.

Per-core pipeline (bf16 compute / f32 accumulate):
  - projections on TensorE fp8 DoubleRow from x^T [128d, 8dc, 512t];
    tanh on ScalarE with sigmoid(z) = 0.5*tanh(z/2)+0.5 folded
  - gate gp = tv*(tu+1): tensor_scalar add (4x) + tensor_tensor mult
    (2x) on VectorE
  - scores IN [128t, 4tc] LAYOUT: 8 stationary-swap matmuls
    (lhsT = gp chunk [128h,128t], rhs = W2 column [128h,1]) -> s4 PSUM
  - exp4 on ScalarE [128,4]; em4 = e4*mask4 + per-partition denominator
    in one tiny stt
  - pooling via NATURAL-layout x [128t, 4tc, 1024d] bf16:
    4 tensor_scalar products x_tc * em4[:,tc] (4x mode), TT add-tree
    (2x) -> wxs [128t, 1024d]; ONE ones-stationary matmul reduces the
    128 token-partitions -> zn [1, 1024] PSUM; ScalarE copies to SBUF
  - no broadcast op, no [1,512] exp/em, no transposed bf16 copy of x
"""

import math

import numpy as np
import ml_dtypes

import concourse.bass as bass
import concourse.bacc as bacc
import concourse.tile as tile
from concourse import mybir, bass_isa
from concourse.bass_utils import run_bass_kernel_spmd

F32 = mybir.dt.float32
BF16 = mybir.dt.bfloat16
F8 = mybir.dt.float8e4
NPF8 = mybir.dt.np(F8)
DR = mybir.MatmulPerfMode.DoubleRow
AF = mybir.ActivationFunctionType
OP = mybir.AluOpType

B, N, D, H = 16, 4096, 1024, 256
NCORES = 8
P = 128                    # partitions
NTOK = 512                 # tokens per work item
NG = N // NTOK             # max items per bag = 8
DC = D // P                # 8 d-chunks
HC = H // P                # 2 h-chunks
TC = NTOK // P             # 4 token-chunks per work item


def build_graph(K):
    nc = bacc.Bacc(None)
    xn_ext = nc.declare_dram_parameter("xN", [K, P, TC, D], BF16, isOutput=False)
    xt8_ext = nc.declare_dram_parameter("xT8", [K, P, DC, NTOK], F8, isOutput=False)
    vw_ext = nc.declare_dram_parameter("Vw", [P, DC, HC, P], F8, isOutput=False)
    uw_ext = nc.declare_dram_parameter("Uw", [P, DC, HC, P], F8, isOutput=False)
    vb_ext = nc.declare_dram_parameter("Vb", [P, HC], F32, isOutput=False)
    ubh_ext = nc.declare_dram_parameter("Ubh", [P, HC], F32, isOutput=False)
    w2_ext = nc.declare_dram_parameter("W2", [P, HC], BF16, isOutput=False)
    mask_ext = nc.declare_dram_parameter("mask4", [K, P, TC], BF16, isOutput=False)
    NB = (K + 2) // 3   # zn batches of up to 3 groups at partitions 0/32/64
    outz_ext = nc.declare_dram_parameter("out_zn", [3, NB, D], F32, isOutput=True)
    outd_ext = nc.declare_dram_parameter("out_den", [P, K], F32, isOutput=True)
    with tile.TileContext(nc) as tc:
        with (
            tc.tile_pool(name="xn", bufs=8) as p_xn,
            tc.tile_pool(name="x8", bufs=5) as p_x8,
            tc.tile_pool(name="act", bufs=3) as p_act,
            tc.tile_pool(name="mk", bufs=8) as p_mk,
            tc.tile_pool(name="small", bufs=3) as p_small,
            tc.tile_pool(name="wx", bufs=3) as p_wx,
            tc.tile_pool(name="one", bufs=1) as p_one,
            tc.tile_pool(name="pproj", bufs=5, space="PSUM") as p_proj,
            tc.tile_pool(name="ps4", bufs=1, space="PSUM") as p_s4,
            tc.tile_pool(name="pzn", bufs=1, space="PSUM") as p_zn,
        ):
            v_sb = p_one.tile([P, DC, HC, P], F8, tag="vw")
            u_sb = p_one.tile([P, DC, HC, P], F8, tag="uw")
            nc.sync.dma_start(out=v_sb, in_=vw_ext[:, :, :, :])
            nc.scalar.dma_start(out=u_sb, in_=uw_ext[:, :, :, :])
            vb_sb = p_one.tile([P, HC], F32, tag="vb")
            ubh_sb = p_one.tile([P, HC], F32, tag="ubh")
            nc.scalar.dma_start(out=vb_sb, in_=vb_ext[:, :])
            nc.scalar.dma_start(out=ubh_sb, in_=ubh_ext[:, :])
            w2_sb = p_one.tile([P, HC], BF16, tag="w2")
            nc.scalar.dma_start(out=w2_sb, in_=w2_ext[:, :])
            ones_sb = p_one.tile([P, 1], BF16, tag="ones")
            nc.vector.memset(ones_sb, 1.0)

            den_sb = p_one.tile([P, K], F32, tag="den")
            zn_sb = p_one.tile([P, NB, D], F32, tag="zn")
            nc.vector.memset(den_sb, 0.0)

            st = {}   # per-group tiles carried across pipeline stages

            def stage_load(k):
                xt8 = p_x8.tile([P, DC, NTOK], F8, tag="xt8", name=f"xt8_{k}")
                nc.sync.dma_start(out=xt8, in_=xt8_ext[k])
                xn = p_xn.tile([P, TC, D], BF16, tag="xn", name=f"xn{k}")
                with tc.tile_wait_until(0.004 * max(k - 1, 0), enable=k < K - 2):
                    nc.scalar.dma_start(out=xn, in_=xn_ext[k])
                mk = p_mk.tile([P, TC], BF16, tag="mk", name=f"mk{k}")
                nc.gpsimd.dma_start(out=mk, in_=mask_ext[k])
                st[k] = {"xt8": xt8, "xn": xn, "mk": mk}

            def stage_proj(k):
                xt8 = st[k]["xt8"]
                tv = p_act.tile([P, HC, NTOK], BF16, tag="tv", name=f"tv{k}")
                tu = p_act.tile([P, HC, NTOK], BF16, tag="tu", name=f"tu{k}")
                for hc in range(HC):
                    psv = p_proj.tile([P, NTOK], F32, tag="proj",
                                      name=f"psv{k}_{hc}")
                    psu = p_proj.tile([P, NTOK], F32, tag="proj",
                                      name=f"psu{k}_{hc}")
                    for j in range(DC // 2):
                        d2 = slice(2 * j, 2 * j + 2)
                        nc.tensor.matmul(psv, v_sb[:, d2, hc, :], xt8[:, d2, :],
                                         start=(j == 0), stop=(j == DC // 2 - 1),
                                         perf_mode=DR)
                    for j in range(DC // 2):
                        d2 = slice(2 * j, 2 * j + 2)
                        nc.tensor.matmul(psu, u_sb[:, d2, hc, :], xt8[:, d2, :],
                                         start=(j == 0), stop=(j == DC // 2 - 1),
                                         perf_mode=DR)
                    # weights pre-scaled by 32 on host for fp8 range
                    nc.scalar.activation(out=tv[:, hc, :], in_=psv, func=AF.Tanh,
                                         bias=vb_sb[:, hc:hc + 1], scale=1.0 / 32)
                    nc.scalar.activation(out=tu[:, hc, :], in_=psu, func=AF.Tanh,
                                         bias=ubh_sb[:, hc:hc + 1], scale=0.5 / 32)
                st[k]["tv"] = tv
                st[k]["tu"] = tu

            def stage_gate(k):
                # g2 = tv*tu; s = sum W2*(tv*tu) + sum W2*tv
                #           = sum w*tanh(xV+b)*sigmoid(xU+b)   (W2 = 0.5*W)
                tv, tu = st[k]["tv"], st[k]["tu"]
                g2 = p_act.tile([P, HC, NTOK], BF16, tag="g", name=f"g{k}")
                nc.vector.tensor_mul(g2, tu, tv)
                st[k]["g2"] = g2

            def stage_score(k):
                g2, tv = st[k]["g2"], st[k]["tv"]
                # scores in [128t, tc] layout: stationary-swap matmuls
                s4 = p_s4.tile([P, TC], F32, tag="s4", name=f"s4_{k}")
                for t in range(TC):
                    srcs = [(tv, 0), (tv, 1), (g2, 0), (g2, 1)]
                    for c, (m, hc) in enumerate(srcs):
                        nc.tensor.matmul(s4[:, t:t + 1],
                                         m[:, hc, t * P:(t + 1) * P],
                                         w2_sb[:, hc:hc + 1],
                                         start=(c == 0), stop=(c == 3))
                e4 = p_small.tile([P, TC], BF16, tag="e4", name=f"e4_{k}")
                nc.scalar.activation(out=e4, in_=s4, func=AF.Exp,
                                     bias=0.0, scale=1.0)
                st[k]["e4"] = e4

            def stage_pool(k):
                e4, mk, xn = st[k]["e4"], st[k]["mk"], st[k]["xn"]
                em4 = p_small.tile([P, TC], F32, tag="em4", name=f"em4_{k}")
                nc.vector.scalar_tensor_tensor(out=em4, in0=e4, scalar=1.0,
                                               in1=mk, op0=OP.mult, op1=OP.mult,
                                               accum_out=den_sb[:, k:k + 1])
                # weighted x: wx_tc = x_nat[:, tc, :] * em4[:, tc]  (TS 4x)
                wx = p_wx.tile([P, TC, D], BF16, tag="wx", name=f"wx{k}")
                for t in range(TC):
                    nc.vector.tensor_scalar_mul(wx[:, t, :], xn[:, t, :],
                                                em4[:, t:t + 1])
                if k < K - 3:
                    # pairwise TT add tree (2x) -> wq [128t, 2, 1024d]
                    wq = p_wx.tile([P, 2, D], BF16, tag="wq", name=f"wq{k}")
                    nc.vector.tensor_add(wq[:, 0, :], wx[:, 0, :], wx[:, 1, :])
                    nc.vector.tensor_add(wq[:, 1, :], wx[:, 2, :], wx[:, 3, :])
                    st[k]["wq"] = wq
                else:
                    st[k]["wx"] = wx

            def stage_zn(k):
                wq = st[k].get("wq")
                s = k % 3   # zn row (partition 32*s) within the current batch
                if s == 0:
                    st["zn_ps"] = p_zn.tile([P, D], F32, tag="znp",
                                            name=f"znp{k // 3}")
                zn_ps = st["zn_ps"]
                # reduce the 128 token-partitions into zn row 32*s
                if wq is not None:
                    nq = 2
                else:
                    wq, nq = st[k]["wx"], TC
                for h in range(2):
                    sl = slice(h * D // 2, (h + 1) * D // 2)
                    for q in range(nq):
                        nc.tensor.matmul(zn_ps[32 * s:32 * s + 1, sl], ones_sb,
                                         wq[:, q, sl], start=(q == 0),
                                         stop=(q == nq - 1))
                if s == 2 or k == K - 1:
                    b = k // 3
                    nc.scalar.activation(out=zn_sb[:, b, :], in_=zn_ps,
                                         func=AF.Copy, bias=0.0, scale=1.0)
                del st[k]

            PIPE = 3
            for k in range(min(PIPE, K)):
                stage_load(k)
            for it in range(K + 4):
                if it + PIPE < K:
                    stage_load(it + PIPE)
                if it < K:
                    stage_proj(it)
                if 0 <= it - 2 < K:
                    with tc.tile_wait_until(0.004 * (it - 1), enable=it < K - 1):
                        stage_score(it - 2)
                if 0 <= it - 3 < K:
                    with tc.tile_wait_until(0.004 * (it - 2) + 0.001,
                                            enable=it < K - 1):
                        stage_pool(it - 3)
                if 0 <= it - 4 < K:
                    with tc.tile_wait_until(0.004 * (it - 3) + 0.002,
                                            enable=it < K - 1):
                        stage_zn(it - 4)
                if it < K:
                    stage_gate(it)

            nc.sync.dma_start(out=outz_ext[:, :, :],
                              in_=zn_sb[0:96:32, :, :])
            nc.gpsimd.dma_start(out=outd_ext[:, :], in_=den_sb)

    nc.finalize()
    return nc


_GRAPHS = {}


def _get_graph(K):
    if K not in _GRAPHS:
        _GRAPHS[K] = build_graph(K)
    return _GRAPHS[K]


def _prep_host(x, lengths, V_w, V_b, U_w, U_b, W_w, W_b):
    lengths = np.maximum(np.asarray(lengths).astype(np.int64), 1)
    groups = np.minimum((lengths + NTOK - 1) // NTOK, NG)
    items = [(b, gi) for b in range(B) for gi in range(int(groups[b]))]
    K = math.ceil(len(items) / NCORES)
    assign = [items[c * K:(c + 1) * K] for c in range(NCORES)]

    def warr(w):  # [D, H] -> [dp, dc, hc, h] fp8, pre-scaled by 32
        return np.ascontiguousarray(
            (w * 32.0).reshape(DC, P, HC, P).transpose(1, 0, 2, 3).astype(NPF8))
    Vw = warr(V_w)
    Uw = warr(U_w)
    Vb = np.ascontiguousarray(V_b.reshape(HC, P).T, dtype=np.float32)
    Ubh = np.ascontiguousarray((U_b * 0.5).reshape(HC, P).T, dtype=np.float32)
    W2 = np.ascontiguousarray(
        (0.5 * W_w[:, 0]).reshape(HC, P).T.astype(ml_dtypes.bfloat16))

    xbf = x.astype(ml_dtypes.bfloat16)  # [B, N, D]
    ar = np.arange(NTOK)

    in_maps = []
    for c in range(NCORES):
        xts = np.zeros((K, P, DC, NTOK), dtype=ml_dtypes.bfloat16)
        xns = np.zeros((K, P, TC, D), dtype=ml_dtypes.bfloat16)
        msk = np.zeros((K, P, TC), dtype=ml_dtypes.bfloat16)
        for k, (b, gi) in enumerate(assign[c]):
            xg = xbf[b, gi * NTOK:(gi + 1) * NTOK, :]        # [512, 1024]
            xts[k] = xg.reshape(NTOK, DC, P).transpose(2, 1, 0)
            xns[k] = xg.reshape(TC, P, D).transpose(1, 0, 2)
            msk[k] = (gi * NTOK + ar < lengths[b]).reshape(TC, P).T
        in_maps.append({"xN": xns, "xT8": xts.astype(NPF8), "mask4": msk,
                        "Vw": Vw, "Uw": Uw,
                        "Vb": Vb, "Ubh": Ubh, "W2": W2})
    return in_maps, assign, K


def kernel(x, lengths, V_w, V_b, U_w, U_b, W_w, W_b, _trace=False, _trace_kwargs=None):
    x = np.asarray(x)
    in_maps, assign, K = _prep_host(
        x, lengths, np.asarray(V_w), np.asarray(V_b), np.asarray(U_w),
        np.asarray(U_b), np.asarray(W_w), np.asarray(W_b),
    )
    nc = _get_graph(K)
    res = run_bass_kernel_spmd(
        nc, in_maps, core_ids=list(range(NCORES)),
        trace=_trace, **(_trace_kwargs or {}),
    )
    z = np.zeros((B, D), dtype=np.float64)
    den = np.zeros((B,), dtype=np.float64)
    for c in range(NCORES):
        zn = np.asarray(res.results[c]["out_zn"], dtype=np.float64)    # [3, NB, D]
        dc_ = np.asarray(res.results[c]["out_den"], dtype=np.float64)  # [P, K]
        for k, (b, gi) in enumerate(assign[c]):
            z[b] += zn[k % 3, k // 3, :]
            den[b] += dc_[:, k].sum()
    den = np.where(den <= 0, 1.0, den)
    out = (z / den[:, None]).astype(np.float32)
    if _trace:
        return out, res
    return out


if __name__ == "__main__":
    rng = np.random.default_rng(0)
    x = rng.standard_normal((B, N, D), dtype=np.float32)
    lengths = rng.integers(0, N, (B,)).astype(np.int32)
    s = 1.0 / np.sqrt(D)
    inputs = dict(
        x=x, lengths=lengths,
        V_w=(rng.standard_normal((D, H), dtype=np.float32) * s),
        V_b=np.zeros(H, np.float32),
        U_w=(rng.standard_normal((D, H), dtype=np.float32) * s),
        U_b=np.zeros(H, np.float32),
        W_w=(rng.standard_normal((H, 1), dtype=np.float32) / 16.0),
        W_b=np.zeros(1, np.float32),
    )
    out = kernel(**inputs)
    print(out.shape, out.dtype)


# revision 27
# speedup vs baseline: 1.1469x; 1.1469x over previous
"""ABMIL gated-attention MIL pooling on 8 TRN2 NeuronCores.

Work-item data parallelism: every 512-token group of every bag is an
independent work item; the ceil(G_tot/8) items per core are balanced
across cores.  Per item (512 tokens, D=1024, H=256):

    A   = tanh(x Vw + Vb) * sigmoid(x Uw + Ub)        [512, H]
    s   = A Ww                                        [512]
    e   = exp(s) * mask                               [512]   (no max-sub:
          |s| <= sum|0.5 W| ~ 13, exp fits f32/bf16 easily)
    zk  = e @ x_group,  dk = sum(e)                   [D], [1]

Host combines: Z_b = (sum_k zk) / (sum_k dk) over the bag's items.
Wb shifts every score equally -> cancels -> dropped.

Per-core pipeline (bf16 compute / f32 accumulate):
  - projections on TensorE fp8 DoubleRow from x^T [128d, 8dc, 512t];
    tanh on ScalarE with sigmoid(z) = 0.5*tanh(z/2)+0.5 folded
  - gate gp = tv*(tu+1): tensor_scalar add (4x) + tensor_tensor mult
    (2x) on VectorE
  - scores IN [128t, 4tc] LAYOUT: 8 stationary-swap matmuls
    (lhsT = gp chunk [128h,128t], rhs = W2 column [128h,1]) -> s4 PSUM
  - exp4 on ScalarE [128,4]; em4 = e4*mask4 + per-partition denominator
    in one tiny stt
  - pooling via NATURAL-layout x [128t, 4tc, 1024d] bf16:
    4 tensor_scalar products x_tc * em4[:,tc] (4x mode), TT add-tree
    (2x) -> wxs [128t, 1024d]; ONE ones-stationary matmul reduces the
    128 token-partitions -> zn [1, 1024] PSUM; ScalarE copies to SBUF
  - no broadcast op, no [1,512] exp/em, no transposed bf16 copy of x
"""

import math

import numpy as np
import ml_dtypes

import concourse.bass as bass
import concourse.bacc as bacc
import concourse.tile as tile
from concourse import mybir, bass_isa
from concourse.bass_utils import run_bass_kernel_spmd

F32 = mybir.dt.float32
BF16 = mybir.dt.bfloat16
F8 = mybir.dt.float8e4
NPF8 = mybir.dt.np(F8)
DR = mybir.MatmulPerfMode.DoubleRow
AF = mybir.ActivationFunctionType
OP = mybir.AluOpType

B, N, D, H = 16, 4096, 1024, 256
NCORES = 8
P = 128                    # partitions
NTOK = 512                 # tokens per work item
NG = N // NTOK             # max items per bag = 8
DC = D // P                # 8 d-chunks
HC = H // P                # 2 h-chunks
TC = NTOK // P             # 4 token-chunks per work item


def build_graph(K):
    nc = bacc.Bacc(None)
    xn_ext = nc.declare_dram_parameter("xN", [K, P, TC, D], BF16, isOutput=False)
    xt8_ext = nc.declare_dram_parameter("xT8", [K, P, DC, NTOK], F8, isOutput=False)
    vw_ext = nc.declare_dram_parameter("Vw", [P, DC, HC, P], F8, isOutput=False)
    uw_ext = nc.declare_dram_parameter("Uw", [P, DC, HC, P], F8, isOutput=False)
    vb_ext = nc.declare_dram_parameter("Vb", [P, HC], F32, isOutput=False)
    ubh_ext = nc.declare_dram_parameter("Ubh", [P, HC], F32, isOutput=False)
    w2_ext = nc.declare_dram_parameter("W2", [P, HC], BF16, isOutput=False)
    mask_ext = nc.declare_dram_parameter("mask4", [K, P, TC], BF16, isOutput=False)
    NB = (K + 2) // 3   # zn batches of up to 3 groups at partitions 0/32/64
    outz_ext = nc.declare_dram_parameter("out_zn", [3, NB, D], F32, isOutput=True)
    outd_ext = nc.declare_dram_parameter("out_den", [P, K], F32, isOutput=True)
    with tile.TileContext(nc) as tc:
        with (
            tc.tile_pool(name="xn", bufs=8) as p_xn,
            tc.tile_pool(name="x8", bufs=5) as p_x8,
            tc.tile_pool(name="act", bufs=3) as p_act,
            tc.tile_pool(name="mk", bufs=8) as p_mk,
            tc.tile_pool(name="small", bufs=3) as p_small,
            tc.tile_pool(name="wx", bufs=3) as p_wx,
            tc.tile_pool(name="one", bufs=1) as p_one,
            tc.tile_pool(name="pproj", bufs=5, space="PSUM") as p_proj,
            tc.tile_pool(name="ps4", bufs=1, space="PSUM") as p_s4,
            tc.tile_pool(name="pzn", bufs=1, space="PSUM") as p_zn,
        ):
            v_sb = p_one.tile([P, DC, HC, P], F8, tag="vw")
            u_sb = p_one.tile([P, DC, HC, P], F8, tag="uw")
            nc.sync.dma_start(out=v_sb, in_=vw_ext[:, :, :, :])
            nc.scalar.dma_start(out=u_sb, in_=uw_ext[:, :, :, :])
            vb_sb = p_one.tile([P, HC], F32, tag="vb")
            ubh_sb = p_one.tile([P, HC], F32, tag="ubh")
            nc.scalar.dma_start(out=vb_sb, in_=vb_ext[:, :])
            nc.scalar.dma_start(out=ubh_sb, in_=ubh_ext[:, :])
            w2_sb = p_one.tile([P, HC], BF16, tag="w2")
            nc.scalar.dma_start(out=w2_sb, in_=w2_ext[:, :])
            ones_sb = p_one.tile([P, 1], BF16, tag="ones")
            nc.vector.memset(ones_sb, 1.0)

            den_sb = p_one.tile([P, K], F32, tag="den")
            zn_sb = p_one.tile([P, NB, D], F32, tag="zn")
            nc.vector.memset(den_sb, 0.0)

            st = {}   # per-group tiles carried across pipeline stages

            def stage_load(k):
                xt8 = p_x8.tile([P, DC, NTOK], F8, tag="xt8", name=f"xt8_{k}")
                nc.sync.dma_start(out=xt8, in_=xt8_ext[k])
                xn = p_xn.tile([P, TC, D], BF16, tag="xn", name=f"xn{k}")
                nc.scalar.dma_start(out=xn, in_=xn_ext[k])
                mk = p_mk.tile([P, TC], BF16, tag="mk", name=f"mk{k}")
                nc.gpsimd.dma_start(out=mk, in_=mask_ext[k])
                st[k] = {"xt8": xt8, "xn": xn, "mk": mk}

            def stage_proj(k):
                xt8 = st[k]["xt8"]
                tv = p_act.tile([P, HC, NTOK], BF16, tag="tv", name=f"tv{k}")
                tu = p_act.tile([P, HC, NTOK], BF16, tag="tu", name=f"tu{k}")
                for hc in range(HC):
                    psv = p_proj.tile([P, NTOK], F32, tag="proj",
                                      name=f"psv{k}_{hc}")
                    psu = p_proj.tile([P, NTOK], F32, tag="proj",
                                      name=f"psu{k}_{hc}")
                    for j in range(DC // 2):
                        d2 = slice(2 * j, 2 * j + 2)
                        nc.tensor.matmul(psv, v_sb[:, d2, hc, :], xt8[:, d2, :],
                                         start=(j == 0), stop=(j == DC // 2 - 1),
                                         perf_mode=DR)
                    for j in range(DC // 2):
                        d2 = slice(2 * j, 2 * j + 2)
                        nc.tensor.matmul(psu, u_sb[:, d2, hc, :], xt8[:, d2, :],
                                         start=(j == 0), stop=(j == DC // 2 - 1),
                                         perf_mode=DR)
                    # weights pre-scaled by 32 on host for fp8 range
                    nc.scalar.activation(out=tv[:, hc, :], in_=psv, func=AF.Tanh,
                                         bias=vb_sb[:, hc:hc + 1], scale=1.0 / 32)
                    nc.scalar.activation(out=tu[:, hc, :], in_=psu, func=AF.Tanh,
                                         bias=ubh_sb[:, hc:hc + 1], scale=0.5 / 32)
                st[k]["tv"] = tv
                st[k]["tu"] = tu

            def stage_gate(k):
                # g2 = tv*tu; s = sum W2*(tv*tu) + sum W2*tv
                #           = sum w*tanh(xV+b)*sigmoid(xU+b)   (W2 = 0.5*W)
                tv, tu = st[k]["tv"], st[k]["tu"]
                g2 = p_act.tile([P, HC, NTOK], BF16, tag="g", name=f"g{k}")
                nc.vector.tensor_mul(g2, tu, tv)
                st[k]["g2"] = g2

            def stage_score(k):
                g2, tv = st[k]["g2"], st[k]["tv"]
                # scores in [128t, tc] layout: stationary-swap matmuls
                s4 = p_s4.tile([P, TC], F32, tag="s4", name=f"s4_{k}")
                for t in range(TC):
                    srcs = [(tv, 0), (tv, 1), (g2, 0), (g2, 1)]
                    for c, (m, hc) in enumerate(srcs):
                        nc.tensor.matmul(s4[:, t:t + 1],
                                         m[:, hc, t * P:(t + 1) * P],
                                         w2_sb[:, hc:hc + 1],
                                         start=(c == 0), stop=(c == 3))
                e4 = p_small.tile([P, TC], BF16, tag="e4", name=f"e4_{k}")
                nc.scalar.activation(out=e4, in_=s4, func=AF.Exp,
                                     bias=0.0, scale=1.0)
                st[k]["e4"] = e4

            def stage_pool(k):
                e4, mk, xn = st[k]["e4"], st[k]["mk"], st[k]["xn"]
                em4 = p_small.tile([P, TC], F32, tag="em4", name=f"em4_{k}")
                nc.vector.scalar_tensor_tensor(out=em4, in0=e4, scalar=1.0,
                                               in1=mk, op0=OP.mult, op1=OP.mult,
                                               accum_out=den_sb[:, k:k + 1])
                # weighted x: wx_tc = x_nat[:, tc, :] * em4[:, tc]  (TS 4x)
                wx = p_wx.tile([P, TC, D], BF16, tag="wx", name=f"wx{k}")
                for t in range(TC):
                    nc.vector.tensor_scalar_mul(wx[:, t, :], xn[:, t, :],
                                                em4[:, t:t + 1])
                if k < K - 3:
                    # pairwise TT add tree (2x) -> wq [128t, 2, 1024d]
                    wq = p_wx.tile([P, 2, D], BF16, tag="wq", name=f"wq{k}")
                    nc.vector.tensor_add(wq[:, 0, :], wx[:, 0, :], wx[:, 1, :])
                    nc.vector.tensor_add(wq[:, 1, :], wx[:, 2, :], wx[:, 3, :])
                    st[k]["wq"] = wq
                else:
                    st[k]["wx"] = wx

            def stage_zn(k):
                wq = st[k].get("wq")
                s = k % 3   # zn row (partition 32*s) within the current batch
                if s == 0:
                    st["zn_ps"] = p_zn.tile([P, D], F32, tag="znp",
                                            name=f"znp{k // 3}")
                zn_ps = st["zn_ps"]
                # reduce the 128 token-partitions into zn row 32*s
                if wq is not None:
                    nq = 2
                else:
                    wq, nq = st[k]["wx"], TC
                for h in range(2):
                    sl = slice(h * D // 2, (h + 1) * D // 2)
                    for q in range(nq):
                        nc.tensor.matmul(zn_ps[32 * s:32 * s + 1, sl], ones_sb,
                                         wq[:, q, sl], start=(q == 0),
                                         stop=(q == nq - 1))
                if s == 2 or k == K - 1:
                    b = k // 3
                    nc.scalar.activation(out=zn_sb[:, b, :], in_=zn_ps,
                                         func=AF.Copy, bias=0.0, scale=1.0)
                del st[k]

            PIPE = 3
            for k in range(min(PIPE, K)):
                stage_load(k)
            for it in range(K + 4):
                if it + PIPE < K:
                    stage_load(it + PIPE)
                if it < K:
                    stage_proj(it)
                if 0 <= it - 2 < K:
                    with tc.tile_wait_until(0.004 * (it - 1), enable=it < K - 1):
                        stage_score(it - 2)
                if 0 <= it - 3 < K:
                    with tc.tile_wait_until(0.004 * (it - 2) + 0.001,
                                            enable=it < K - 1):
                        stage_pool(it - 3)
                if 0 <= it - 4 < K:
                    with tc.tile_wait_until(0.004 * (it - 3) + 0.002,
                                            enable=it < K - 1):
                        stage_zn(it - 4)
                if it < K:
                    stage_gate(it)

            nc.sync.dma_start(out=outz_ext[:, :, :],
                              in_=zn_sb[0:96:32, :, :])
            nc.gpsimd.dma_start(out=outd_ext[:, :], in_=den_sb)

    nc.finalize()
    return nc


_GRAPHS = {}


def _get_graph(K):
    if K not in _GRAPHS:
        _GRAPHS[K] = build_graph(K)
    return _GRAPHS[K]


def _prep_host(x, lengths, V_w, V_b, U_w, U_b, W_w, W_b):
    lengths = np.maximum(np.asarray(lengths).astype(np.int64), 1)
    groups = np.minimum((lengths + NTOK - 1) // NTOK, NG)
    items = [(b, gi) for b in range(B) for gi in range(int(groups[b]))]
    K = math.ceil(len(items) / NCORES)
    assign = [items[c * K:(c + 1) * K] for c in range(NCORES)]

    def warr(w):  # [D, H] -> [dp, dc, hc, h] fp8, pre-scaled by 32
        return np.ascontiguousarray(
            (w * 32.0).reshape(DC, P, HC, P).transpose(1, 0, 2, 3).astype(NPF8))
    Vw = warr(V_w)
    Uw = warr(U_w)
    Vb = np.ascontiguousarray(V_b.reshape(HC, P).T, dtype=np.float32)
    Ubh = np.ascontiguousarray((U_b * 0.5).reshape(HC, P).T, dtype=np.float32)
    W2 = np.ascontiguousarray(
        (0.5 * W_w[:, 0]).reshape(HC, P).T.astype(ml_dtypes.bfloat16))

    xbf = x.astype(ml_dtypes.bfloat16)  # [B, N, D]
    ar = np.arange(NTOK)

    in_maps = []
    for c in range(NCORES):
        xts = np.zeros((K, P, DC, NTOK), dtype=ml_dtypes.bfloat16)
        xns = np.zeros((K, P, TC, D), dtype=ml_dtypes.bfloat16)
        msk = np.zeros((K, P, TC), dtype=ml_dtypes.bfloat16)
        for k, (b, gi) in enumerate(assign[c]):
            xg = xbf[b, gi * NTOK:(gi + 1) * NTOK, :]        # [512, 1024]
            xts[k] = xg.reshape(NTOK, DC, P).transpose(2, 1, 0)
            xns[k] = xg.reshape(TC, P, D).transpose(1, 0, 2)
            msk[k] = (gi * NTOK + ar < lengths[b]).reshape(TC, P).T
        in_maps.append({"xN": xns, "xT8": xts.astype(NPF8), "mask4": msk,
                        "Vw": Vw, "Uw": Uw,
                        "Vb": Vb, "Ubh": Ubh, "W2": W2})
    return in_maps, assign, K


def kernel(x, lengths, V_w, V_b, U_w, U_b, W_w, W_b, _trace=False, _trace_kwargs=None):
    x = np.asarray(x)
    in_maps, assign, K = _prep_host(
        x, lengths, np.asarray(V_w), np.asarray(V_b), np.asarray(U_w),
        np.asarray(U_b), np.asarray(W_w), np.asarray(W_b),
    )
    nc = _get_graph(K)
    res = run_bass_kernel_spmd(
        nc, in_maps, core_ids=list(range(NCORES)),
        trace=_trace, **(_trace_kwargs or {}),
    )
    z = np.zeros((B, D), dtype=np.float64)
    den = np.zeros((B,), dtype=np.float64)
    for c in range(NCORES):
        zn = np.asarray(res.results[c]["out_zn"], dtype=np.float64)    # [3, NB, D]
        dc_ = np.asarray(res.results[c]["out_den"], dtype=np.float64)  # [P, K]
        for k, (b, gi) in enumerate(assign[c]):
            z[b] += zn[k % 3, k // 3, :]
            den[b] += dc_[:, k].sum()
    den = np.where(den <= 0, 1.0, den)
    out = (z / den[:, None]).astype(np.float32)
    if _trace:
        return out, res
    return out


if __name__ == "__main__":
    rng = np.random.default_rng(0)
    x = rng.standard_normal((B, N, D), dtype=np.float32)
    lengths = rng.integers(0, N, (B,)).astype(np.int32)
    s = 1.0 / np.sqrt(D)
    inputs = dict(
        x=x, lengths=lengths,
        V_w=(rng.standard_normal((D, H), dtype=np.float32) * s),
        V_b=np.zeros(H, np.float32),
        U_w=(rng.standard_normal((D, H), dtype=np.float32) * s),
        U_b=np.zeros(H, np.float32),
        W_w=(rng.standard_normal((H, 1), dtype=np.float32) / 16.0),
        W_b=np.zeros(1, np.float32),
    )
    out = kernel(**inputs)
    print(out.shape, out.dtype)
